# revision 1
# baseline (speedup 1.0000x reference)
"""Multi-head attention (B=2, L=2048, D=1024, H=16, RoPE, softmax, out-proj)
on 8 Trainium2 NeuronCores.

Sharding: 2-way data parallel on batch x 4-way tensor parallel on heads.
Core c handles batch c//4 and heads 4*(c%4) .. 4*(c%4)+3.

v4: software-pipelined head loop.  On this toolchain every matmul pays
a serial LDWEIGHTS unless it sits in a dependency-free run (then the PE
pulls the load ahead), so the emission order keeps the PE packed with
ready work:
  - per head-half hh, the k-loop emits QK^T (zero-padded K^T stationary,
    FWL) + exp for hh interleaved with the P^T-stationary PV chains of
    head hh-1 (whose pt tiles are all ready -> chains run back-to-back
    with hidden weight loads).  The scalar engine's exp stream never has
    to wait for a PV phase to finish.
  - o~[q,65] = P^T.T @ [V | 1] accumulated over k in PSUM (col 64 = the
    softmax denominator); normalized straight out of PSUM with a cheap
    per-partition reciprocal, transposed per q-tile at the last head.
  - out-proj + ReduceScatter + output DMA run per 512-query chunk (4
    chunks), so only the last chunk's collective is exposed; the last
    chunk's collective is further split into two 256-column pieces.
  - V projection is woven into head 0's k-loop so it fills the PE while
    the first exps run.

All matmuls bf16 with fp32 PSUM accumulation; softmax in fp32 (PSUM)
with bf16 P storage.  Host reassembles the full [2, 2048, 1024] output.
"""

import numpy as np
import ml_dtypes
from contextlib import ExitStack

import concourse.bass as bass
import concourse.tile as tile
from concourse import bacc, mybir
from concourse.bass_utils import run_bass_kernel_spmd
from concourse.masks import make_identity

BF16 = mybir.dt.bfloat16
F32 = mybir.dt.float32

B, L, D = 2, 2048, 1024
H_TOT, H = 16, 4          # total heads, heads per core
HD, HF = 64, 32           # head dim, rope freqs
DL = H * HD               # local head dims per core = 256
P = 128
KT = L // P               # 16 k-tiles
DK = D // P               # 8 contraction tiles over model dim
CH = 512                  # collective chunk (queries)
QH = L // 2               # L-half
ROPE_BASE = 10000.0
GROUPS = [[0, 1, 2, 3], [4, 5, 6, 7]]

_CACHED_NC = None


def _build_program():
    nc = bacc.Bacc("TRN2", target_bir_lowering=False, debug=False, num_devices=8)

    xT_ext = nc.dram_tensor("xT", [DK, 2, P, QH], BF16, kind="ExternalInput")
    wqk_ext = nc.dram_tensor("wqkT", [DK, P, 4 * P], BF16, kind="ExternalInput")
    wv_ext = nc.dram_tensor("wvT", [DK, P, DL], BF16, kind="ExternalInput")
    wo_ext = nc.dram_tensor("woT", [2, P, D], BF16, kind="ExternalInput")
    cos_ext = nc.dram_tensor("cosF", [P, L], F32, kind="ExternalInput")
    sin_ext = nc.dram_tensor("sinF", [P, L], F32, kind="ExternalInput")
    out_ext = nc.dram_tensor("out", [DL, L], BF16, kind="ExternalOutput")

    partials = [nc.dram_tensor(f"partialT{c}", [D, CH], BF16) for c in range(4)]
    scats = [nc.dram_tensor(f"scatT{c}", [DL, CH], BF16) for c in range(4)]

    with tile.TileContext(nc) as tc:
        with ExitStack() as ctx:
            pers = ctx.enter_context(tc.tile_pool(name="pers", bufs=1))

            wv = pers.tile([P, DK, DL], BF16, tag="wv")
            wo = pers.tile([P, 2, D], BF16, tag="wo")
            qt = [pers.tile([P, 2, QH], BF16, tag=f"qt{i}", name=f"qt{i}")
                  for i in range(2)]                       # head-contig Q^T, per L-half
            ktz = [pers.tile([P, H, QH], BF16, tag=f"ktz{i}", name=f"ktz{i}")
                   for i in range(2)]                      # zero-padded K^T, per L-half
            v1 = pers.tile([P, KT, H * (HD + 1)], BF16, tag="v1")  # [V | 1]
            ident = pers.tile([P, P], BF16, tag="ident")

            xp = ctx.enter_context(tc.tile_pool(name="xp", bufs=1))
            xt = [[None, None] for _ in range(DK)]
            for dk in range(DK):
                for cp in range(2):
                    xt[dk][cp] = xp.tile([P, QH], BF16, tag=f"xt{dk}_{cp}",
                                         name=f"x_t{dk}_{cp}")

            # ---------------- QK projection + rope ----------------
            with ExitStack() as pctx:
                pj = pctx.enter_context(tc.tile_pool(name="proj", bufs=1))
                tmp = pctx.enter_context(tc.tile_pool(name="ptmp", bufs=4))
                pp = pctx.enter_context(tc.tile_pool(name="pjps", bufs=1, space="PSUM"))

                wqk = [pj.tile([P, 4 * P], BF16, tag=f"wqk{dk}", name=f"wqk{dk}")
                       for dk in range(DK)]
                cosf = pj.tile([P, L], F32, tag="cosf")
                sinf = pj.tile([P, L], F32, tag="sinf")
                qkr = [pj.tile([P, 4, QH], BF16, tag=f"qkr{i}", name=f"qkr{i}")
                       for i in range(2)]  # qr1 qr2 kr1 kr2, per L-half

                # warm the ACT exp table during the load ramp (the table
                # DMA otherwise fires lazily before the first real exp)
                warm = tmp.tile([P, 1], F32, tag="t1", name="warm")
                warm2 = tmp.tile([P, 1], F32, tag="t2", name="warm2")
                nc.vector.memset(warm[:], 0.0)
                nc.scalar.activation(warm2[:], warm[:],
                                     mybir.ActivationFunctionType.Exp)

                # load order = need order; interleave weights with x so the
                # first projection chain starts after ~2 transfers
                for dk in range(DK):
                    nc.sync.dma_start(out=wqk[dk][:], in_=wqk_ext[dk])
                    nc.sync.dma_start(out=xt[dk][0][:], in_=xT_ext[dk, 0])
                nc.sync.dma_start(out=cosf[:], in_=cos_ext[:])
                nc.sync.dma_start(out=sinf[:], in_=sin_ext[:])
                nc.vector.memset(ktz[0][:], 0.0)
                nc.vector.memset(ktz[1][:], 0.0)
                for dk in range(DK):
                    nc.sync.dma_start(out=xt[dk][1][:], in_=xT_ext[dk, 1])
                for dk in range(DK):
                    nc.sync.dma_start(out=wv[:, dk, :], in_=wv_ext[dk])
                for t in range(2):
                    nc.sync.dma_start(out=wo[:, t, :], in_=wo_ext[t])
                make_identity(nc, ident[:])

                # m: 0=qx1 1=qx2 2=kx1 3=kx2; m-outer so early chains
                # finish early and rope starts sooner
                for cp in range(2):
                    pq = {(c, m): pp.tile([P, CH], F32, tag=f"qk{c}{m}",
                                          name=f"pq_{cp}_{c}_{m}")
                          for c in range(2) for m in range(4)}
                    for c in range(2):
                        for m in range(4):
                            for dk in range(DK):
                                nc.tensor.matmul(
                                    pq[(c, m)][:],
                                    wqk[dk][:, m * P:(m + 1) * P],
                                    xt[dk][cp][:, c * CH:(c + 1) * CH],
                                    start=(dk == 0), stop=(dk == DK - 1))
                    for c in range(2):
                        xs = slice((2 * cp + c) * CH, (2 * cp + c + 1) * CH)
                        ws = slice(c * CH, (c + 1) * CH)
                        for base in (0, 2):
                            x1, x2 = pq[(c, base)], pq[(c, base + 1)]
                            t1 = tmp.tile([P, CH], F32, tag="t1")
                            t2 = tmp.tile([P, CH], F32, tag="t2")
                            nc.vector.tensor_mul(t1[:], x1[:], cosf[:, xs])
                            nc.vector.tensor_mul(t2[:], x2[:], sinf[:, xs])
                            nc.vector.tensor_sub(qkr[cp][:, base, ws], t1[:], t2[:])
                            t3 = tmp.tile([P, CH], F32, tag="t1")
                            t4 = tmp.tile([P, CH], F32, tag="t2")
                            nc.vector.tensor_mul(t3[:], x1[:], sinf[:, xs])
                            nc.vector.tensor_mul(t4[:], x2[:], cosf[:, xs])
                            nc.vector.tensor_add(qkr[cp][:, base + 1, ws], t3[:], t4[:])

                for lh in range(2):
                    for h in range(H):
                        t, pb = h // 2, 64 * (h % 2)
                        hs = slice(32 * h, 32 * h + 32)
                        nc.sync.dma_start(out=qt[lh][pb:pb + 32, t, :], in_=qkr[lh][hs, 0, :])
                        nc.sync.dma_start(out=qt[lh][pb + 32:pb + 64, t, :], in_=qkr[lh][hs, 1, :])
                        nc.sync.dma_start(out=ktz[lh][pb:pb + 32, h, :], in_=qkr[lh][hs, 2, :])
                        nc.sync.dma_start(out=ktz[lh][pb + 32:pb + 64, h, :], in_=qkr[lh][hs, 3, :])

            # ------------- attention, head-pipelined + finish -------------
            with ExitStack() as actx:
                ptp = actx.enter_context(tc.tile_pool(name="ptp", bufs=1))
                fin = actx.enter_context(tc.tile_pool(name="fin", bufs=1))
                aps = actx.enter_context(tc.tile_pool(name="aps", bufs=1, space="PSUM"))

                o_nrm = {}   # lh -> [P, QH//P, DL] tile
                onT = {}     # lh -> [P, 2, QH] tile

                def emit_qkt_exp(hh, k, pts):
                    """QK^T + exp for (head-half hh, k-tile k)."""
                    lh, h = hh // 4, hh % 4
                    st = aps.tile([P, QH], F32, tag="st", bufs=2)
                    for qc in range(2):
                        cs = slice(qc * CH, (qc + 1) * CH)
                        nc.tensor.matmul(
                            st[:, cs], ktz[k // 8][:, h, (k % 8) * P:(k % 8 + 1) * P],
                            qt[lh][:, h // 2, cs],
                            start=True, stop=True, skip_group_check=True)
                    pt = ptp.tile([P, QH], BF16, tag="pt", bufs=34, name="pt")
                    nc.scalar.activation(pt[:], st[:],
                                         mybir.ActivationFunctionType.Exp)
                    pts.append(pt)

                def emit_vproj(k):
                    """V projection for k-tile k (woven into head 0)."""
                    pv = aps.tile([P, DL], F32, tag="misc", bufs=2, name="pv")
                    for dk in range(DK):
                        nc.tensor.matmul(
                            pv[:], xt[dk][k // 8][:, (k % 8) * P:(k % 8 + 1) * P],
                            wv[:, dk, :],
                            start=(dk == 0), stop=(dk == DK - 1),
                            skip_group_check=True)
                    src3 = pv[:].rearrange("p (h d) -> p h d", h=H)
                    dst3 = v1[:, k, :].rearrange("p (h d) -> p h d", h=H)
                    nc.vector.tensor_copy(dst3[:, :, 0:HD], src3)
                    nc.vector.memset(dst3[:, :, HD:HD + 1], 1.0)

                def emit_pv_qtile(hh, q, pts):
                    """PV chain + normalize for q-tile q of head-half hh."""
                    lh, h = hh // 4, hh % 4
                    vs = slice(h * (HD + 1), (h + 1) * (HD + 1))
                    ob = aps.tile([P, HD + 1], F32, tag="ob", bufs=2)
                    for k in range(KT):
                        nc.tensor.matmul(
                            ob[:], pts[k][:, q * P:(q + 1) * P], v1[:, k, vs],
                            start=(k == 0), stop=(k == KT - 1),
                            skip_group_check=True)
                    rec = fin.tile([P, 1], F32, tag="rec", bufs=4)
                    nc.vector.reciprocal(rec[:], ob[:, HD:HD + 1])
                    nc.vector.tensor_scalar(
                        out=o_nrm[lh][:, q, h * HD:(h + 1) * HD],
                        in0=ob[:, 0:HD], scalar1=rec[:], scalar2=None,
                        op0=mybir.AluOpType.mult)
                    if h == H - 1:
                        # last head: transpose this completed q-tile
                        for t in range(2):
                            ptr = aps.tile([P, P], BF16, tag="misc", bufs=2,
                                           name="ptr")
                            nc.tensor.transpose(
                                ptr[:], o_nrm[lh][:, q, t * P:(t + 1) * P],
                                ident[:])
                            nc.vector.tensor_copy(
                                onT[lh][:, t, q * P:(q + 1) * P], ptr[:])

                def emit_op2(lh, qc, j):
                    """Two out-proj column tiles (2j, 2j+1) of chunk qc."""
                    gc = 2 * lh + qc
                    cols = slice(qc * CH, (qc + 1) * CH)
                    for ot in (2 * j, 2 * j + 1):
                        po = aps.tile([P, CH], F32, tag="misc", bufs=2, name="po")
                        for t in range(2):
                            nc.tensor.matmul(
                                po[:], wo[:, t, ot * P:(ot + 1) * P],
                                onT[lh][:, t, cols],
                                start=(t == 0), stop=(t == 1),
                                skip_group_check=True)
                        so = fin.tile([P, CH], BF16, tag="so", bufs=4, name="so")
                        nc.vector.tensor_copy(so[:], po[:])
                        nc.sync.dma_start(
                            out=partials[gc][ot * P:(ot + 1) * P, :], in_=so[:])

                def emit_rs(lh, qc):
                    gc = 2 * lh + qc
                    nc.gpsimd.collective_compute(
                        "ReduceScatter", mybir.AluOpType.add,
                        replica_groups=GROUPS,
                        ins=[partials[gc][:]], outs=[scats[gc][:]])
                    # on the sync queue, not gpsimd: an out-DMA between two
                    # tail collectives would delay the second one's start
                    nc.sync.dma_start(
                        out=out_ext[:, gc * CH:(gc + 1) * CH], in_=scats[gc][:])

                def emit_finish(lh, qc):
                    for j in range(DK // 2):
                        emit_op2(lh, qc, j)
                    emit_rs(lh, qc)

                pts_prev = None
                for hh in range(8):
                    lh, h = hh // 4, hh % 4
                    if h == 0:
                        o_nrm[lh] = fin.tile([P, QH // P, DL], BF16,
                                             tag="onrm", bufs=2, name="onrm")
                        onT[lh] = fin.tile([P, 2, QH], BF16, tag="onT",
                                           bufs=2, name="onT")
                    pts = []
                    for k in range(KT):
                        if hh == 0:
                            # vproj first so the PE has ready work while
                            # the first QK^T waits on the qt/ktz shuffle
                            emit_vproj(k)
                        emit_qkt_exp(hh, k, pts)
                        if hh > 0 and k % 2 == 1:
                            # PV of the previous head trails: one q-tile
                            # chain per odd k-step
                            q = (k - 1) // 2
                            emit_pv_qtile(hh - 1, q, pts_prev)
                            # half 0 finish work spread 2 column-tiles per
                            # step so the exp stream is never starved
                            if hh == 4 and q >= 4:
                                emit_op2(0, 0, q - 4)
                                if q == 7:
                                    emit_rs(0, 0)
                            elif hh == 5 and q <= 3:
                                emit_op2(0, 1, q)
                                if q == 3:
                                    emit_rs(0, 1)
                    pts_prev = pts
                # drain: last head's PV + chunk finishes for half 1
                for q in range(QH // P):
                    emit_pv_qtile(7, q, pts_prev)
                    if q == 3:
                        emit_finish(1, 0)
                emit_finish(1, 1)

    nc.compile()
    return nc


def _prep_inputs(x, W_qkv, W_out):
    """Host-side sharding / layout prep -> per-core input maps."""
    Wq, Wk, Wv = W_qkv[0:D], W_qkv[D:2 * D], W_qkv[2 * D:3 * D]
    inv = 1.0 / (ROPE_BASE ** (np.arange(0, HD, 2, dtype=np.float64) / HD))
    pos = np.arange(L, dtype=np.float64)
    ang = pos[:, None] * inv[None, :]                     # [L, 32]
    cosF = np.tile(np.cos(ang).T, (H, 1)).astype(np.float32)  # [128, L]
    sinF = np.tile(np.sin(ang).T, (H, 1)).astype(np.float32)

    scale = float(HD) ** -0.5
    in_maps = []
    for c in range(8):
        b, g = c // 4, c % 4
        rows_x1 = np.array([64 * (4 * g + h) + 2 * f for h in range(H) for f in range(HF)])
        rows_x2 = rows_x1 + 1
        wqkT = np.concatenate([
            (scale * Wq[rows_x1]).T, (scale * Wq[rows_x2]).T,
            Wk[rows_x1].T, Wk[rows_x2].T], axis=1)        # [1024, 512]
        wvT = Wv[DL * g:DL * (g + 1)].T                   # [1024, 256]
        woT = W_out[:, DL * g:DL * (g + 1)].T             # [256, 1024]
        xTt = x[b].T.reshape(DK, P, 2, QH).transpose(0, 2, 1, 3)
        in_maps.append({
            "xT": np.ascontiguousarray(xTt).astype(ml_dtypes.bfloat16),
            "wqkT": np.ascontiguousarray(wqkT.reshape(DK, P, 4 * P)).astype(ml_dtypes.bfloat16),
            "wvT": np.ascontiguousarray(wvT.reshape(DK, P, DL)).astype(ml_dtypes.bfloat16),
            "woT": np.ascontiguousarray(woT.reshape(2, P, D)).astype(ml_dtypes.bfloat16),
            "cosF": cosF, "sinF": sinF,
        })
    return in_maps


def _run(in_maps, trace=False):
    global _CACHED_NC
    if _CACHED_NC is None:
        _CACHED_NC = _build_program()
    kw = dict(trace=True) if trace else {}
    return run_bass_kernel_spmd(_CACHED_NC, in_maps, list(range(8)), **kw)


def kernel(x, W_qkv, W_out, _trace=False):
    x = np.asarray(x, dtype=np.float32)
    W_qkv = np.asarray(W_qkv, dtype=np.float32)
    W_out = np.asarray(W_out, dtype=np.float32)
    res = _run(_prep_inputs(x, W_qkv, W_out), trace=_trace)
    out = np.empty((B, L, D), dtype=np.float32)
    for b in range(B):
        outT = np.concatenate([res.results[4 * b + j]["out"] for j in range(4)], axis=0)
        out[b] = outT.T
    if _trace:
        kernel.last_exec_time_ns = res.exec_time_ns
        kernel.last_trace = res.instructions_and_trace
    return out



# revision 2
# speedup vs baseline: 1.3050x; 1.3050x over previous
"""Multi-head attention (B=2, L=2048, D=1024, H=16, RoPE, softmax, out-proj)
on 8 Trainium2 NeuronCores.

Sharding: 2-way data parallel on batch x 4-way tensor parallel on heads.
Core c handles batch c//4 and heads 4*(c%4) .. 4*(c%4)+3.

v5: collective-free.  Each core writes its out-proj partial [D, L] to
DRAM; the host does the 4-way reduction per batch (the all-reduce of the
hint) during unshard.  This removes the ReduceScatter tail (~45us) and
the sync-queue backpressure its out-DMAs created mid-kernel.

Pipeline (same core idea as v4, retuned):
  - QKV projection with stationary-weight reuse (m,dk outer; c inner) so
    each LDWEIGHTS serves two 512-col streams.
  - rope per L-half on DVE; the qt/ktz shuffle DMA is issued per L-half
    right after its rope so head 0's QK^T can start early.
  - per head-half hh, the k-loop emits QK^T (zero-padded K^T stationary)
    + exp interleaved with the P^T-stationary PV chains of head hh-1.
    The scalar engine's exp stream (~137us total) paces the loop.
  - o~[q,65] = P^T.T @ [V | 1] accumulated over k in PSUM (col 64 = the
    softmax denominator); normalized out of PSUM with a per-partition
    reciprocal, transposed per q-tile at the last head.
  - out-proj runs per 512-query chunk, spread two column-tiles per
    pipeline step (hh 4/5 for L-half 0, drain q-steps for L-half 1), so
    only the final chunk's 8 matmuls + DMA are exposed.
  - V projection is woven into head 0's k-loop so it fills the PE while
    the first exps run.

All matmuls bf16 with fp32 PSUM accumulation; softmax in fp32 (PSUM)
with bf16 P storage; cos/sin tables in bf16.  Host reassembles + reduces
the full [2, 2048, 1024] output.
"""

import numpy as np
import ml_dtypes
from contextlib import ExitStack

import concourse.bass as bass
import concourse.tile as tile
from concourse import bacc, mybir
from concourse.bass_utils import run_bass_kernel_spmd
from concourse.masks import make_identity

BF16 = mybir.dt.bfloat16
F32 = mybir.dt.float32

B, L, D = 2, 2048, 1024
H_TOT, H = 16, 4          # total heads, heads per core
HD, HF = 64, 32           # head dim, rope freqs
DL = H * HD               # local head dims per core = 256
P = 128
KT = L // P               # 16 k-tiles
DK = D // P               # 8 contraction tiles over model dim
CH = 512                  # out-proj chunk (queries)
QH = L // 2               # L-half
ROPE_BASE = 10000.0

_CACHED_NC = None


def _build_program():
    nc = bacc.Bacc("TRN2", target_bir_lowering=False, debug=False, num_devices=8)

    xT_ext = nc.dram_tensor("xT", [DK, 2, P, QH], BF16, kind="ExternalInput")
    wqk_ext = nc.dram_tensor("wqkT", [DK, P, 4 * P], BF16, kind="ExternalInput")
    wv_ext = nc.dram_tensor("wvT", [DK, P, DL], BF16, kind="ExternalInput")
    wo_ext = nc.dram_tensor("woT", [2, P, D], BF16, kind="ExternalInput")
    cos_ext = nc.dram_tensor("cosF", [P, L], BF16, kind="ExternalInput")
    sin_ext = nc.dram_tensor("sinF", [P, L], BF16, kind="ExternalInput")
    out_ext = nc.dram_tensor("out", [D, L], BF16, kind="ExternalOutput")

    with tile.TileContext(nc) as tc:
        with ExitStack() as ctx:
            pers = ctx.enter_context(tc.tile_pool(name="pers", bufs=1))

            wv = pers.tile([P, DK, DL], BF16, tag="wv")
            wo = pers.tile([P, 2, D], BF16, tag="wo")
            qt = [pers.tile([P, 2, QH], BF16, tag=f"qt{i}", name=f"qt{i}")
                  for i in range(2)]                       # head-contig Q^T, per L-half
            ktz = [pers.tile([P, H, QH], BF16, tag=f"ktz{i}", name=f"ktz{i}")
                   for i in range(2)]                      # zero-padded K^T, per L-half
            v1 = pers.tile([P, KT, H * (HD + 1)], BF16, tag="v1")  # [V | 1]
            ident = pers.tile([P, P], BF16, tag="ident")

            xp = ctx.enter_context(tc.tile_pool(name="xp", bufs=1))
            xt = [[None, None] for _ in range(DK)]
            for dk in range(DK):
                for cp in range(2):
                    xt[dk][cp] = xp.tile([P, QH], BF16, tag=f"xt{dk}_{cp}",
                                         name=f"x_t{dk}_{cp}")

            # ---------------- QK projection + rope ----------------
            with ExitStack() as pctx:
                pj = pctx.enter_context(tc.tile_pool(name="proj", bufs=1))
                tmp = pctx.enter_context(tc.tile_pool(name="ptmp", bufs=4))
                pp = pctx.enter_context(tc.tile_pool(name="pjps", bufs=1, space="PSUM"))

                wqk = [pj.tile([P, 4 * P], BF16, tag=f"wqk{dk}", name=f"wqk{dk}")
                       for dk in range(DK)]
                cosf = pj.tile([P, L], BF16, tag="cosf")
                sinf = pj.tile([P, L], BF16, tag="sinf")
                qkr = [pj.tile([P, 4, QH], BF16, tag=f"qkr{i}", name=f"qkr{i}")
                       for i in range(2)]  # qr1 qr2 kr1 kr2, per L-half

                # warm the ACT exp table during the load ramp (the table
                # DMA otherwise fires lazily before the first real exp)
                warm = tmp.tile([P, 1], F32, tag="t1", name="warm")
                warm2 = tmp.tile([P, 1], F32, tag="t2", name="warm2")
                nc.vector.memset(warm[:], 0.0)
                nc.scalar.activation(warm2[:], warm[:],
                                     mybir.ActivationFunctionType.Exp)

                # load order = need order; interleave weights with x so the
                # first projection chain starts after ~2 transfers
                for dk in range(DK):
                    nc.sync.dma_start(out=wqk[dk][:], in_=wqk_ext[dk])
                    nc.sync.dma_start(out=xt[dk][0][:], in_=xT_ext[dk, 0])
                nc.sync.dma_start(out=cosf[:], in_=cos_ext[:])
                nc.sync.dma_start(out=sinf[:], in_=sin_ext[:])
                for dk in range(DK):
                    nc.sync.dma_start(out=wv[:, dk, :], in_=wv_ext[dk])
                nc.vector.memset(ktz[0][:], 0.0)
                nc.vector.memset(ktz[1][:], 0.0)
                for dk in range(DK):
                    nc.sync.dma_start(out=xt[dk][1][:], in_=xT_ext[dk, 1])
                for t in range(2):
                    nc.sync.dma_start(out=wo[:, t, :], in_=wo_ext[t])
                make_identity(nc, ident[:])

                # m: 0=qx1 1=qx2 2=kx1 3=kx2.  (m, dk) outer with c inner:
                # each stationary wqk column block serves both 512-query
                # streams, and the two c-chains interleave so the PE always
                # has a dependency-free next matmul to prefetch weights for.
                for cp in range(2):
                    pq = {(c, m): pp.tile([P, CH], F32, tag=f"qk{c}{m}",
                                          name=f"pq_{cp}_{c}_{m}")
                          for c in range(2) for m in range(4)}
                    for m in range(4):
                        for dk in range(DK):
                            for c in range(2):
                                nc.tensor.matmul(
                                    pq[(c, m)][:],
                                    wqk[dk][:, m * P:(m + 1) * P],
                                    xt[dk][cp][:, c * CH:(c + 1) * CH],
                                    start=(dk == 0), stop=(dk == DK - 1))
                    for c in range(2):
                        xs = slice((2 * cp + c) * CH, (2 * cp + c + 1) * CH)
                        ws = slice(c * CH, (c + 1) * CH)
                        for base in (0, 2):
                            x1, x2 = pq[(c, base)], pq[(c, base + 1)]
                            t1 = tmp.tile([P, CH], F32, tag="t1")
                            t2 = tmp.tile([P, CH], F32, tag="t2")
                            nc.vector.tensor_mul(t1[:], x1[:], cosf[:, xs])
                            nc.vector.tensor_mul(t2[:], x2[:], sinf[:, xs])
                            nc.vector.tensor_sub(qkr[cp][:, base, ws], t1[:], t2[:])
                            t3 = tmp.tile([P, CH], F32, tag="t1")
                            t4 = tmp.tile([P, CH], F32, tag="t2")
                            nc.vector.tensor_mul(t3[:], x1[:], sinf[:, xs])
                            nc.vector.tensor_mul(t4[:], x2[:], cosf[:, xs])
                            nc.vector.tensor_add(qkr[cp][:, base + 1, ws], t3[:], t4[:])
                    # shuffle this L-half into matmul layouts right away so
                    # head 0's QK^T isn't gated on the cp=1 rope
                    for h in range(H):
                        t, pb = h // 2, 64 * (h % 2)
                        hs = slice(32 * h, 32 * h + 32)
                        nc.sync.dma_start(out=qt[cp][pb:pb + 32, t, :], in_=qkr[cp][hs, 0, :])
                        nc.sync.dma_start(out=qt[cp][pb + 32:pb + 64, t, :], in_=qkr[cp][hs, 1, :])
                        nc.sync.dma_start(out=ktz[cp][pb:pb + 32, h, :], in_=qkr[cp][hs, 2, :])
                        nc.sync.dma_start(out=ktz[cp][pb + 32:pb + 64, h, :], in_=qkr[cp][hs, 3, :])

            # ------------- attention, head-pipelined + finish -------------
            with ExitStack() as actx:
                ptp = actx.enter_context(tc.tile_pool(name="ptp", bufs=1))
                fin = actx.enter_context(tc.tile_pool(name="fin", bufs=1))
                aps = actx.enter_context(tc.tile_pool(name="aps", bufs=1, space="PSUM"))

                o_nrm = {}   # lh -> [P, QH//P, DL] tile
                onT = {}     # lh -> [P, 2, QH] tile

                def emit_qkt_exp(hh, k, pts):
                    """QK^T + exp for (head-half hh, k-tile k)."""
                    lh, h = hh // 4, hh % 4
                    st = aps.tile([P, QH], F32, tag="st", bufs=2)
                    for qc in range(2):
                        cs = slice(qc * CH, (qc + 1) * CH)
                        nc.tensor.matmul(
                            st[:, cs], ktz[k // 8][:, h, (k % 8) * P:(k % 8 + 1) * P],
                            qt[lh][:, h // 2, cs],
                            start=True, stop=True, skip_group_check=True)
                    pt = ptp.tile([P, QH], BF16, tag="pt", bufs=34, name="pt")
                    nc.scalar.activation(pt[:], st[:],
                                         mybir.ActivationFunctionType.Exp)
                    pts.append(pt)

                def emit_vproj(k):
                    """V projection for k-tile k (woven into head 0)."""
                    pv = aps.tile([P, DL], F32, tag="misc", bufs=2, name="pv")
                    for dk in range(DK):
                        nc.tensor.matmul(
                            pv[:], xt[dk][k // 8][:, (k % 8) * P:(k % 8 + 1) * P],
                            wv[:, dk, :],
                            start=(dk == 0), stop=(dk == DK - 1),
                            skip_group_check=True)
                    src3 = pv[:].rearrange("p (h d) -> p h d", h=H)
                    dst3 = v1[:, k, :].rearrange("p (h d) -> p h d", h=H)
                    nc.vector.tensor_copy(dst3[:, :, 0:HD], src3)
                    nc.vector.memset(dst3[:, :, HD:HD + 1], 1.0)

                def emit_pv_qtile(hh, q, pts):
                    """PV chain + normalize for q-tile q of head-half hh."""
                    lh, h = hh // 4, hh % 4
                    vs = slice(h * (HD + 1), (h + 1) * (HD + 1))
                    ob = aps.tile([P, HD + 1], F32, tag="ob", bufs=2)
                    for k in range(KT):
                        nc.tensor.matmul(
                            ob[:], pts[k][:, q * P:(q + 1) * P], v1[:, k, vs],
                            start=(k == 0), stop=(k == KT - 1),
                            skip_group_check=True)
                    rec = fin.tile([P, 1], F32, tag="rec", bufs=4)
                    nc.vector.reciprocal(rec[:], ob[:, HD:HD + 1])
                    nc.vector.tensor_scalar(
                        out=o_nrm[lh][:, q, h * HD:(h + 1) * HD],
                        in0=ob[:, 0:HD], scalar1=rec[:], scalar2=None,
                        op0=mybir.AluOpType.mult)
                    if h == H - 1:
                        # last head: transpose this completed q-tile
                        for t in range(2):
                            ptr = aps.tile([P, P], BF16, tag="misc", bufs=2,
                                           name="ptr")
                            nc.tensor.transpose(
                                ptr[:], o_nrm[lh][:, q, t * P:(t + 1) * P],
                                ident[:])
                            nc.vector.tensor_copy(
                                onT[lh][:, t, q * P:(q + 1) * P], ptr[:])

                def emit_op2(lh, qc, j):
                    """Two out-proj column tiles (2j, 2j+1) of chunk qc."""
                    gc = 2 * lh + qc
                    cols = slice(qc * CH, (qc + 1) * CH)
                    for ot in (2 * j, 2 * j + 1):
                        po = aps.tile([P, CH], F32, tag="misc", bufs=2, name="po")
                        for t in range(2):
                            nc.tensor.matmul(
                                po[:], wo[:, t, ot * P:(ot + 1) * P],
                                onT[lh][:, t, cols],
                                start=(t == 0), stop=(t == 1),
                                skip_group_check=True)
                        so = fin.tile([P, CH], BF16, tag="so", bufs=4, name="so")
                        nc.vector.tensor_copy(so[:], po[:])
                        nc.sync.dma_start(
                            out=out_ext[ot * P:(ot + 1) * P,
                                        gc * CH:(gc + 1) * CH], in_=so[:])

                pts_prev = None
                for hh in range(8):
                    lh, h = hh // 4, hh % 4
                    if h == 0:
                        o_nrm[lh] = fin.tile([P, QH // P, DL], BF16,
                                             tag="onrm", bufs=2, name="onrm")
                        onT[lh] = fin.tile([P, 2, QH], BF16, tag="onT",
                                           bufs=2, name="onT")
                    pts = []
                    for k in range(KT):
                        if hh == 0:
                            # vproj first so the PE has ready work while
                            # the first QK^T waits on the qt/ktz shuffle
                            emit_vproj(k)
                        emit_qkt_exp(hh, k, pts)
                        if hh > 0 and k % 2 == 1:
                            # PV of the previous head trails: one q-tile
                            # chain per odd k-step
                            q = (k - 1) // 2
                            emit_pv_qtile(hh - 1, q, pts_prev)
                            # half 0 out-proj spread 2 column-tiles per
                            # step so the exp stream is never starved
                            if hh == 4 and q >= 4:
                                emit_op2(0, 0, q - 4)
                            elif hh == 5 and q <= 3:
                                emit_op2(0, 1, q)
                    pts_prev = pts
                # drain: last head's PV + out-proj chunks for half 1,
                # spread so only the last chunk's matmuls are exposed
                for q in range(QH // P):
                    emit_pv_qtile(7, q, pts_prev)
                    if q >= 4:
                        emit_op2(1, 0, q - 4)
                for j in range(DK // 2):
                    emit_op2(1, 1, j)

    nc.compile()
    return nc


def _prep_inputs(x, W_qkv, W_out):
    """Host-side sharding / layout prep -> per-core input maps."""
    Wq, Wk, Wv = W_qkv[0:D], W_qkv[D:2 * D], W_qkv[2 * D:3 * D]
    inv = 1.0 / (ROPE_BASE ** (np.arange(0, HD, 2, dtype=np.float64) / HD))
    pos = np.arange(L, dtype=np.float64)
    ang = pos[:, None] * inv[None, :]                     # [L, 32]
    cosF = np.tile(np.cos(ang).T, (H, 1)).astype(ml_dtypes.bfloat16)  # [128, L]
    sinF = np.tile(np.sin(ang).T, (H, 1)).astype(ml_dtypes.bfloat16)

    scale = float(HD) ** -0.5
    in_maps = []
    for c in range(8):
        b, g = c // 4, c % 4
        rows_x1 = np.array([64 * (4 * g + h) + 2 * f for h in range(H) for f in range(HF)])
        rows_x2 = rows_x1 + 1
        wqkT = np.concatenate([
            (scale * Wq[rows_x1]).T, (scale * Wq[rows_x2]).T,
            Wk[rows_x1].T, Wk[rows_x2].T], axis=1)        # [1024, 512]
        wvT = Wv[DL * g:DL * (g + 1)].T                   # [1024, 256]
        woT = W_out[:, DL * g:DL * (g + 1)].T             # [256, 1024]
        xTt = x[b].T.reshape(DK, P, 2, QH).transpose(0, 2, 1, 3)
        in_maps.append({
            "xT": np.ascontiguousarray(xTt).astype(ml_dtypes.bfloat16),
            "wqkT": np.ascontiguousarray(wqkT.reshape(DK, P, 4 * P)).astype(ml_dtypes.bfloat16),
            "wvT": np.ascontiguousarray(wvT.reshape(DK, P, DL)).astype(ml_dtypes.bfloat16),
            "woT": np.ascontiguousarray(woT.reshape(2, P, D)).astype(ml_dtypes.bfloat16),
            "cosF": cosF, "sinF": sinF,
        })
    return in_maps


def _run(in_maps, trace=False):
    global _CACHED_NC
    if _CACHED_NC is None:
        _CACHED_NC = _build_program()
    kw = dict(trace=True) if trace else {}
    return run_bass_kernel_spmd(_CACHED_NC, in_maps, list(range(8)), **kw)


def kernel(x, W_qkv, W_out, _trace=False):
    x = np.asarray(x, dtype=np.float32)
    W_qkv = np.asarray(W_qkv, dtype=np.float32)
    W_out = np.asarray(W_out, dtype=np.float32)
    res = _run(_prep_inputs(x, W_qkv, W_out), trace=_trace)
    out = np.empty((B, L, D), dtype=np.float32)
    for b in range(B):
        # host-side all-reduce of the 4 tensor-parallel partials
        acc = np.zeros((D, L), dtype=np.float32)
        for j in range(4):
            acc += np.asarray(res.results[4 * b + j]["out"], dtype=np.float32)
        out[b] = acc.T
    if _trace:
        kernel.last_exec_time_ns = res.exec_time_ns
        kernel.last_trace = res.instructions_and_trace
    return out
